# revision 1
# baseline (speedup 1.0000x reference)
"""Multi-head self-attention kernel for Trainium2 (8 NeuronCores, Bass/Tile).

See build_core_program for the per-core dataflow.  Sharding: 8 cores =
2 batches x 4 head-groups; each core computes one batch and 4 heads end to
end (no collectives), host sums the 4 bf16 partial out-projections per batch
and folds in the v-bias correction.
"""


import numpy as np
import ml_dtypes
import sys

try:
    import concourse.bass as bass
except ImportError:  # pragma: no cover
    sys.path.insert(0, "/opt/trn_rl_repo")
    import concourse.bass as bass

import concourse.bacc as bacc
import concourse.mybir as mybir
import concourse.tile as tile
from concourse.bass_utils import run_bass_kernel_spmd

BF16 = mybir.dt.bfloat16
F32 = mybir.dt.float32
AF = mybir.ActivationFunctionType

D_MODEL = 1024
HEADS_PER_CORE = 4
HEAD_DIM = 64
CH = HEADS_PER_CORE * HEAD_DIM  # 256


def build_core_program(S=2048, D=D_MODEL, reps=1, use_gpsimd_bc=True,
                       use_fast_recip=False):
    nc = bacc.Bacc(trn_type="TRN2", target_bir_lowering=False, debug=False,
                   enable_partition_id=False)

    xT_d = nc.dram_tensor("xT", [D, S], BF16, kind="ExternalInput").ap()
    wq_d = nc.dram_tensor("wq", [D, CH], BF16, kind="ExternalInput").ap()
    wk_d = nc.dram_tensor("wk", [D, CH], BF16, kind="ExternalInput").ap()
    wv_d = nc.dram_tensor("wv", [D, CH], BF16, kind="ExternalInput").ap()
    wo_d = nc.dram_tensor("wo", [CH, D], BF16, kind="ExternalInput").ap()
    bqk_d = nc.dram_tensor("bqk", [4, 128, 1], F32, kind="ExternalInput").ap()
    out_d = nc.dram_tensor("out", [S, D], BF16, kind="ExternalOutput").ap()

    NT = S // 128     # key tiles
    ND = D // 128     # d_model contraction chunks
    NW = S // 1024    # query windows
    assert NW == 2 and NT == 16

    with tile.TileContext(nc) as tc:
        with (
            tc.tile_pool(name="persist", bufs=1) as persist,
            tc.tile_pool(name="probs", bufs=42) as probs_pool,
            tc.tile_pool(name="bc", bufs=3) as bc_pool,
            tc.tile_pool(name="outb", bufs=2) as outb_pool,
            tc.tile_pool(name="ps_sc", bufs=3, space="PSUM") as ps_sc,
            tc.tile_pool(name="ps_v", bufs=2, space="PSUM") as ps_v,
        ):
            # --- constants ------------------------------------------------
            xT = [persist.tile([128, S], BF16, name=f"xT{i}", tag=f"xT{i}")
                  for i in range(ND)]
            wq = [persist.tile([128, CH], BF16, name=f"wq{i}", tag=f"wq{i}")
                  for i in range(ND)]
            wk = [persist.tile([128, CH], BF16, name=f"wk{i}", tag=f"wk{i}")
                  for i in range(ND)]
            wv = [persist.tile([128, CH], BF16, name=f"wv{i}", tag=f"wv{i}")
                  for i in range(ND)]
            bias = [persist.tile([128, 1], F32, name=f"bias{i}", tag=f"bias{i}")
                    for i in range(4)]
            # critical path first (xT+wq+wk feed the first qk chains), split
            # across the two HWDGE trigger engines (SP + ACT)
            for i in range(ND):
                nc.scalar.dma_start(wq[i], wq_d[128 * i:128 * (i + 1), :])
            for i in range(4):
                nc.sync.dma_start(xT[i], xT_d[128 * i:128 * (i + 1), :])
            for i in range(4, ND):
                nc.scalar.dma_start(xT[i], xT_d[128 * i:128 * (i + 1), :])
            for i in range(ND):
                nc.sync.dma_start(wk[i], wk_d[128 * i:128 * (i + 1), :])
            for i in range(4):
                nc.scalar.dma_start(bias[i], bqk_d[i])
            for i in range(ND):
                nc.sync.dma_start(wv[i], wv_d[128 * i:128 * (i + 1), :])
            wo = [persist.tile([128, D], BF16, name=f"wo{i}", tag=f"wo{i}")
                  for i in range(2)]
            for i in range(2):
                nc.sync.dma_start(wo[i], wo_d[128 * i:128 * (i + 1), :])

            # dependency-free ACT warmup (loads the exp table early)
            warm = persist.tile([128, 1], F32, name="warm", tag="warm")
            nc.vector.memset(warm, 0.0)
            nc.scalar.activation(warm, warm, AF.Exp, bias=0.0, scale=1.0)

            qkT = [persist.tile([128, S], BF16, name=f"qkT{i}", tag=f"qkT{i}")
                   for i in range(4)]
            vsb = [persist.tile([128, HEADS_PER_CORE * 65], BF16,
                                name=f"v{i}", tag=f"v{i}") for i in range(NT)]
            vals = [persist.tile([128, S], BF16, name=f"vals{i}",
                                 tag=f"vals{i}") for i in range(2)]
            # 16 reciprocal rows, all on partition 0 in distinct column
            # slots (partition-offset reads/writes of 1-row APs proved
            # unreliable on HW for both DVE and gpsimd)
            rrs_t = persist.tile([1, 16 * 512], F32, name="rrs", tag="rrs")

            def rrs_row(r):
                return rrs_t[:, 512 * r:512 * (r + 1)]
            ones_col = persist.tile([1, 64], F32, name="ones_col",
                                    tag="ones_col")
            nc.vector.memset(ones_col, 1.0)

            # --- helpers --------------------------------------------------
            def qk_chain(ct, c):
                """qkT[ct][:, 512c:512c+512] = (W.T @ x) + bias."""
                wsrc = wq if ct < 2 else wk
                wcol = (ct % 2) * 128
                ps = ps_sc.tile([128, 512], F32, name="ps_qk", tag="ps_sc")
                for dc in range(ND):
                    nc.tensor.matmul(
                        ps,
                        lhsT=wsrc[dc][:, wcol:wcol + 128],
                        rhs=xT[dc][:, 512 * c:512 * (c + 1)],
                        start=(dc == 0),
                        stop=(dc == ND - 1),
                    )
                nc.vector.tensor_scalar_add(
                    qkT[ct][:, 512 * c:512 * (c + 1)], ps, bias[ct])

            def v_chain(t):
                """vsb[t]: token-major v for tile t, ones col per head.

                Uses ps_v slots: in window (0,0) no values accumulate yet,
                so both ps_v slots are free for these chains."""
                ps = ps_v.tile([128, CH], F32, name="ps_v0", tag="psv")
                for dc in range(ND):
                    nc.tensor.matmul(
                        ps,
                        lhsT=xT[dc][:, 128 * t:128 * (t + 1)],
                        rhs=wv[dc],
                        start=(dc == 0),
                        stop=(dc == ND - 1),
                    )
                nc.vector.memset(vsb[t], 1.0)
                nc.vector.tensor_copy(
                    vsb[t].rearrange("p (h c) -> p h c", c=65)[:, :, 0:64],
                    ps.rearrange("p (h c) -> p h c", c=64),
                )

            def scores_exp(p, w, j):
                """Row-packed pair scores + exp -> (probsA, probsB)."""
                psA = ps_sc.tile([128, 1024], F32, name="ps_scA", tag="ps_sc")
                psB = ps_sc.tile([128, 1024], F32, name="ps_scB", tag="ps_sc")
                for ic in range(2):
                    o = 1024 * w + 512 * ic
                    nc.tensor.matmul(
                        psA[:, 512 * ic:512 * (ic + 1)],
                        lhsT=qkT[2 + p][0:64, 128 * j:128 * (j + 1)],
                        rhs=qkT[p][0:64, o:o + 512],
                        start=True, stop=True,
                    )
                    nc.tensor.matmul(
                        psB[:, 512 * ic:512 * (ic + 1)],
                        lhsT=qkT[2 + p][64:128, 128 * j:128 * (j + 1)],
                        rhs=qkT[p][64:128, o:o + 512],
                        start=True, stop=True,
                    )
                pA = probs_pool.tile([128, 1024], BF16, name="probsA",
                                     tag="probs")
                pB = probs_pool.tile([128, 1024], BF16, name="probsB",
                                     tag="probs")
                nc.scalar.activation(pA, psA, AF.Exp, bias=0.0, scale=0.125)
                nc.scalar.activation(pB, psB, AF.Exp, bias=0.0, scale=0.125)
                return pA, pB

            def val_mm(p, hh, j, ptile, half, psV):
                h = HEADS_PER_CORE * 0 + 2 * p + hh
                nc.tensor.matmul(
                    psV,
                    lhsT=vsb[j][:, 65 * h:65 * h + 65],
                    rhs=ptile[:, 512 * half:512 * (half + 1)],
                    start=(j == 0),
                    stop=(j == NT - 1),
                )

            def drain(p, hh, w, half, psV):
                """psV [65,512] -> normalized vals quadrant."""
                r = 8 * p + 4 * w + 2 * half + hh
                if use_fast_recip:
                    nc.vector.reciprocal_approx_fast(rrs_row(r), psV[64:65, :])
                else:
                    nc.vector.reciprocal(rrs_row(r), psV[64:65, :])
                bc = bc_pool.tile([64, 512], F32, name="bc", tag="bc")
                if use_gpsimd_bc:
                    nc.gpsimd.partition_broadcast(bc, rrs_row(r))
                else:
                    psbc = ps_sc.tile([128, 512], F32, name="ps_bc",
                                      tag="ps_sc")
                    nc.tensor.matmul(psbc[0:64, :], lhsT=ones_col,
                                     rhs=rrs_row(r), start=True, stop=True)
                    nc.vector.tensor_copy(bc, psbc[0:64, :])
                qo = 1024 * w + 512 * half
                nc.vector.tensor_mul(
                    vals[p][64 * hh:64 * hh + 64, qo:qo + 512],
                    psV[0:64, :], bc)

            def outproj(t, act_ok=False):
                ob = outb_pool.tile([128, D], BF16, name="outb", tag="outb")
                for mh in range(2):
                    ps = ps_sc.tile([128, 512], F32, name="ps_out",
                                    tag="ps_sc")
                    for p in range(2):
                        nc.tensor.matmul(
                            ps,
                            lhsT=vals[p][:, 128 * t:128 * (t + 1)],
                            rhs=wo[p][:, 512 * mh:512 * (mh + 1)],
                            start=(p == 0),
                            stop=(p == 1),
                        )
                    # ACT does the copy only when exps are done (tail);
                    # mid-window it would steal from the pacing engine
                    if mh == 1 and act_ok:
                        nc.scalar.activation(ob[:, 512:1024], ps, AF.Copy,
                                             bias=0.0, scale=1.0)
                    else:
                        nc.vector.tensor_copy(
                            ob[:, 512 * mh:512 * (mh + 1)], ps)
                nc.sync.dma_start(out_d[128 * t:128 * (t + 1), :], ob)

            # --- main program --------------------------------------------
            # Window order: (p,w) = (0,0),(0,1),(1,0),(1,1).  All values
            # matmuls for window i run "pass-2 style" during window i+1
            # (half 0 in steps 0-7, half 1 in steps 8-15, two j's per step),
            # through the two ps_v slots.  Window (0,0) instead hosts the
            # v-phase chains in those slots.  qk chains for later windows are
            # spread where their psum slot + PE slack exist.
            def window_values(pp, pw, half, jjs, psVh, probs_store):
                """A few j's of values for (pp, pw, half)."""
                for jj in jjs:
                    prA, prB = probs_store[(pp, pw, jj)]
                    val_mm(pp, 0, jj, prA, half, psVh[0])
                    val_mm(pp, 1, jj, prB, half, psVh[1])

            # values j's per step: front-loaded so the previous window's
            # probs tiles release early (the probs pool is the SBUF-limited
            # resource): half 0 at steps 2-5 (4 j's each), half 1 at steps
            # 6-11 (3,3,3,3,2,2).  The 2-step lead-in keeps the next
            # window's scores from queuing behind val_mms that wait on the
            # previous drain chain to release the ps_v slots.
            VAL_SCHED = {}
            for s in range(2, 6):
                VAL_SCHED[s] = (0, list(range(4 * (s - 2), 4 * (s - 2) + 4)))
            _h1 = [3, 3, 3, 3, 2, 2]
            _off = 0
            for i, n in enumerate(_h1):
                VAL_SCHED[6 + i] = (1, list(range(_off, _off + n)))
                _off += n

            for _rep in range(reps):
                qk_chain(0, 0)
                qk_chain(0, 1)
                qk_chain(2, 0)

                probs_store = {}
                windows = [(0, 0), (0, 1), (1, 0), (1, 1)]
                for wi, (p, w) in enumerate(windows):
                    pp, pw = windows[wi - 1] if wi > 0 else (None, None)
                    psVh = None
                    for j in range(NT):
                        # late qk chains, placed just ahead of first use
                        if (p, w) == (0, 0):
                            v_chain(j)
                            if j in (1, 5, 9):
                                qk_chain(2, 1 + (j - 1) // 4)
                            elif j in (11, 13):
                                qk_chain(0, 2 + (j - 11) // 2)
                        elif (p, w) == (0, 1):
                            if j in (0, 2, 4, 6):
                                qk_chain(1, j // 2)
                            elif j in (8, 10, 12, 14):
                                qk_chain(3, (j - 8) // 2)
                        if pp is not None:
                            if j == 2:
                                psVh = [ps_v.tile([65, 512], F32, name="psVa",
                                                  tag="psv") for _ in range(2)]
                            if j == 6:
                                psVh = [ps_v.tile([65, 512], F32, name="psVb",
                                                  tag="psv") for _ in range(2)]
                        pA, pB = scores_exp(p, w, j)
                        probs_store[(p, w, j)] = (pA, pB)
                        if pp is not None and j in VAL_SCHED:
                            half, jjs = VAL_SCHED[j]
                            window_values(pp, pw, half, jjs, psVh,
                                          probs_store)
                            if j == 5:
                                drain(pp, 0, pw, 0, psVh[0])
                                drain(pp, 1, pw, 0, psVh[1])
                            elif j == 11:
                                drain(pp, 0, pw, 1, psVh[0])
                                drain(pp, 1, pw, 1, psVh[1])
                        # early out-proj: token tiles 0-3 need only query
                        # chunk 0 = half 0 of windows (p,0); pair1's drains
                        # at step 5 of this window
                        if (p, w) == (1, 1) and 8 <= j <= 14 and j % 2 == 0:
                            outproj((j - 8) // 2)

                # tail: values for the last window (both halves), with the
                # remaining out-proj tiles interleaved as their query chunks
                # drain
                for half in range(2):
                    psVh = [ps_v.tile([65, 512], F32, name="psVt", tag="psv")
                            for _ in range(2)]
                    for step in range(8):
                        window_values(1, NW - 1, half,
                                      (2 * step, 2 * step + 1), psVh,
                                      probs_store)
                    drain(1, 0, NW - 1, half, psVh[0])
                    drain(1, 1, NW - 1, half, psVh[1])
                    if half == 0:
                        for t in range(4, 8):   # chunk 1, ready since j=15
                            outproj(t, act_ok=True)
                for t in range(8, 12):          # chunk 2 = last-window half 0
                    outproj(t, act_ok=True)
                for t in range(12, NT):         # chunk 3 = last-window half 1
                    outproj(t, act_ok=True)

    nc.compile()
    return nc


def make_in_maps(x, W_qkv, b_qkv, W_out, n_cores=8):
    """Per-core input dict: core c -> batch c//4, head group c%4."""
    bf = ml_dtypes.bfloat16
    in_maps = []
    for c in range(n_cores):
        b, g = divmod(c, 4)
        heads = range(HEADS_PER_CORE * g, HEADS_PER_CORE * (g + 1))
        qs = np.concatenate([W_qkv[:, 192 * h:192 * h + 64] for h in heads], 1)
        ks = np.concatenate([W_qkv[:, 192 * h + 64:192 * h + 128] for h in heads], 1)
        vs = np.concatenate([W_qkv[:, 192 * h + 128:192 * h + 192] for h in heads], 1)
        bq = np.concatenate([b_qkv[192 * h:192 * h + 64] for h in heads])
        bk = np.concatenate([b_qkv[192 * h + 64:192 * h + 128] for h in heads])
        in_maps.append({
            "xT": np.ascontiguousarray(x[b].T).astype(bf),
            "wq": np.ascontiguousarray(qs).astype(bf),
            "wk": np.ascontiguousarray(ks).astype(bf),
            "wv": np.ascontiguousarray(vs).astype(bf),
            "wo": np.ascontiguousarray(W_out[CH * g:CH * (g + 1)]).astype(bf),
            "bqk": np.stack([bq[:128], bq[128:], bk[:128], bk[128:]])
                     .reshape(4, 128, 1).astype(np.float32),
        })
    return in_maps


_PROGRAM_CACHE = {}


def _get_program(S):
    if S not in _PROGRAM_CACHE:
        _PROGRAM_CACHE[S] = build_core_program(S=S)
    return _PROGRAM_CACHE[S]


class PjrtRunner:
    """Reusable compiled SPMD executable (no donation, so it can be re-run
    back-to-back on device-resident inputs for timing)."""

    def __init__(self, nc, n_cores=8):
        import jax
        from jax.sharding import Mesh, PartitionSpec
        from jax.experimental.shard_map import shard_map
        from concourse import bass2jax, mybir as mb

        bass2jax.install_neuronx_cc_hook()
        self.nc = nc
        self.n_cores = n_cores
        in_names, out_names, out_avals, zero_outs = [], [], [], []
        for alloc in nc.m.functions[0].allocations:
            if not isinstance(alloc, mb.MemoryLocationSet):
                continue
            name = alloc.memorylocations[0].name
            if alloc.kind == "ExternalInput":
                in_names.append(name)
            elif alloc.kind == "ExternalOutput":
                out_names.append(name)
                shape = tuple(alloc.tensor_shape)
                dtype = mb.dt.np(alloc.dtype)
                out_avals.append(jax.core.ShapedArray(shape, dtype))
                zero_outs.append(np.zeros(shape, dtype))
        self.in_names = list(in_names)
        self.out_names = out_names
        self.out_avals = out_avals
        self.zero_outs = zero_outs
        n_params = len(in_names)
        all_names = in_names + out_names

        def _body(*args):
            outs = bass2jax._bass_exec_p.bind(
                *args,
                out_avals=tuple(out_avals),
                in_names=tuple(all_names),
                out_names=tuple(out_names),
                lowering_input_output_aliases=(),
                sim_require_finite=True,
                sim_require_nnan=True,
                nc=nc,
            )
            return tuple(outs)

        devices = jax.devices()[:n_cores]
        self.mesh = Mesh(np.asarray(devices), ("core",))
        in_specs = (PartitionSpec("core"),) * (n_params + len(out_names))
        out_specs = (PartitionSpec("core"),) * len(out_names)
        self.fn = jax.jit(
            shard_map(_body, mesh=self.mesh, in_specs=in_specs,
                      out_specs=out_specs, check_rep=False),
            keep_unused=True,
        )
        self._dev_args = None

    def stage(self, in_maps):
        """Concatenate per-core inputs, upload once, keep device arrays."""
        import jax
        from jax.sharding import NamedSharding, PartitionSpec
        n = self.n_cores
        concat = [
            np.concatenate([np.asarray(in_maps[c][k]) for c in range(n)], axis=0)
            for k in self.in_names
        ]
        concat += [
            np.zeros((n * z.shape[0], *z.shape[1:]), z.dtype)
            for z in self.zero_outs
        ]
        sh = NamedSharding(self.mesh, PartitionSpec("core"))
        self._dev_args = [jax.device_put(a, sh) for a in concat]

    def run(self):
        outs = self.fn(*self._dev_args)
        # keep device arrays for reuse; pull results to host
        res = []
        for c in range(self.n_cores):
            res.append({
                name: np.asarray(outs[i]).reshape(
                    self.n_cores, *self.out_avals[i].shape)[c]
                for i, name in enumerate(self.out_names)
            })
        return res

    def time_iters(self, iters=20):
        import time
        import jax
        outs = self.fn(*self._dev_args)
        jax.block_until_ready(outs)
        t0 = time.perf_counter()
        for _ in range(iters):
            outs = self.fn(*self._dev_args)
        jax.block_until_ready(outs)
        t1 = time.perf_counter()
        return (t1 - t0) / iters


_RUNNER_CACHE = {}


def get_runner(S):
    if S not in _RUNNER_CACHE:
        _RUNNER_CACHE[S] = PjrtRunner(_get_program(S))
    return _RUNNER_CACHE[S]


def combine_outputs(results, W_qkv, b_qkv, W_out, b_out, B, S, D):
    b_v = np.concatenate([b_qkv[192 * h + 128:192 * h + 192] for h in range(16)])
    corr = (b_v.astype(np.float64) @ W_out.astype(np.float64)).astype(np.float32)
    corr += b_out
    out = np.zeros((B, S, D), np.float32)
    for c in range(8):
        out[c // 4] += results[c]["out"].astype(np.float32)
    out += corr[None, None, :]
    return out


def kernel(x, W_qkv, b_qkv, W_out, b_out):
    x = np.asarray(x)
    W_qkv = np.asarray(W_qkv)
    b_qkv = np.asarray(b_qkv)
    W_out = np.asarray(W_out)
    b_out = np.asarray(b_out)
    B, S, D = x.shape

    runner = get_runner(S)
    runner.stage(make_in_maps(x, W_qkv, b_qkv, W_out))
    results = runner.run()
    return combine_outputs(results, W_qkv, b_qkv, W_out, b_out, B, S, D)



# revision 36
# speedup vs baseline: 1.3520x; 1.3520x over previous
"""Multi-head self-attention kernel for Trainium2 (8 NeuronCores, Bass/Tile).

Sharding: 8 cores = 2 batches x 4 head-groups; each core computes one batch
and 4 heads end to end (no collectives); host sums the 4 bf16 partial
out-projections per batch and folds in the v-bias + out-bias correction.

Per-core dataflow (v2):
  - qkT[ct] = (W.T @ x) + bias, ct in {q01,q23,k01,k24}, [128, S] each
    (two heads stacked on partitions 0:64 / 64:128).
  - scores for (pair p, 512-query window w, key tile j): ONE [128, 1024]
    PSUM tile per j holding both heads side by side; ONE exp per j on ACT.
  - values flipped vs v1: out[q, d] with probs as stationary (N=65 moving
    cols incl a ones column that yields the softmax denominator), PSUM-
    accumulated over all 16 key tiles -> half the tensor-engine rows.
  - normalize on DVE via per-partition reciprocal + tensor_scalar_mul,
    pack two heads into a [128, 128] tile, DMA-XBAR transpose into the
    [d, S] vals layout consumed by the out-projection.
  - out-projection: 16 token tiles, 2x2 accumulated matmuls each, DVE
    copies, DMA out.
Window order: (0,0),(0,1),(1,0),(1,1),(0,2),(0,3),(1,2),(1,3); values of
window i run during window i+3/i+2 (front) or i+1 (back) to keep the PE
fed at the ACT exp pace; out-projections trail once both pairs of a
query window have been transposed.
"""


import numpy as np
import ml_dtypes
import sys

try:
    import concourse.bass as bass
except ImportError:  # pragma: no cover
    sys.path.insert(0, "/opt/trn_rl_repo")
    import concourse.bass as bass

import concourse.bacc as bacc
import concourse.mybir as mybir
import concourse.tile as tile
from concourse.bass_utils import run_bass_kernel_spmd

BF16 = mybir.dt.bfloat16
F32 = mybir.dt.float32
AF = mybir.ActivationFunctionType

D_MODEL = 1024
HEADS_PER_CORE = 4
HEAD_DIM = 64
CH = HEADS_PER_CORE * HEAD_DIM  # 256


def build_core_program(S=2048, D=D_MODEL, reps=1):
    nc = bacc.Bacc(trn_type="TRN2", target_bir_lowering=False, debug=False,
                   enable_partition_id=False)

    xT_d = nc.dram_tensor("xT", [D, S], BF16, kind="ExternalInput").ap()
    wq_d = nc.dram_tensor("wq", [D, CH], BF16, kind="ExternalInput").ap()
    wk_d = nc.dram_tensor("wk", [D, CH], BF16, kind="ExternalInput").ap()
    wv_d = nc.dram_tensor("wv", [D, CH], BF16, kind="ExternalInput").ap()
    wo_d = nc.dram_tensor("wo", [CH, D], BF16, kind="ExternalInput").ap()
    bqk_d = nc.dram_tensor("bqk", [4, 128, 1], F32, kind="ExternalInput").ap()
    out_d = nc.dram_tensor("out", [S, D], BF16, kind="ExternalOutput").ap()

    NT = S // 128     # key tiles
    ND = D // 128     # d_model contraction chunks
    NQ = S // 512     # query windows per pair
    assert NT == 16 and NQ == 4 and ND == 8

    with tile.TileContext(nc) as tc:
        with (
            tc.tile_pool(name="persist", bufs=1) as persist,
            tc.tile_pool(name="probs", bufs=56) as probs_pool,
            tc.tile_pool(name="valsb", bufs=6) as valsb_pool,
            tc.tile_pool(name="rec", bufs=6) as rec_pool,
            tc.tile_pool(name="outb", bufs=2) as outb_pool,
            tc.tile_pool(name="ps_sc", bufs=2, space="PSUM") as ps_sc,
            tc.tile_pool(name="ps_mm", bufs=2, space="PSUM") as ps_mm,
            tc.tile_pool(name="ps_val", bufs=2, space="PSUM") as ps_val,
        ):
            # --- constants ------------------------------------------------
            # consolidated SBUF images: one strided DMA each instead of
            # per-chunk DMAs (HWDGE trigger overhead dominates small DMAs)
            xT_all = persist.tile([128, ND * S], BF16, name="xT", tag="xT")
            wq_all = persist.tile([128, ND * CH], BF16, name="wq", tag="wq")
            wk_all = persist.tile([128, ND * CH], BF16, name="wk", tag="wk")
            wv_all = persist.tile([128, ND * CH], BF16, name="wv", tag="wv")
            bias_all = persist.tile([128, 4], F32, name="bias", tag="bias")
            wo_all = persist.tile([128, 2 * D], BF16, name="wo", tag="wo")

            def xTs(dc, c0, c1):
                return xT_all[:, S * dc + c0:S * dc + c1]

            xT_dr = xT_d.rearrange("(dc p) s -> p dc s", p=128)
            xT_sb = xT_all.rearrange("p (dc s) -> p dc s", s=S)
            # priority order: pieces needed by the first qk chains first;
            # all triggers on SP so the ACT sequencer stays free for exps
            nc.sync.dma_start(
                bias_all.rearrange("p b -> p b ()"),
                bqk_d.rearrange("b p one -> p b one"))
            nc.sync.dma_start(
                wq_all.rearrange("p (dc ch) -> p dc ch", ch=CH),
                wq_d.rearrange("(dc p) ch -> p dc ch", p=128))
            nc.sync.dma_start(xT_sb[:, 0:4, 0:512], xT_dr[:, 0:4, 0:512])
            nc.sync.dma_start(
                wk_all.rearrange("p (dc ch) -> p dc ch", ch=CH),
                wk_d.rearrange("(dc p) ch -> p dc ch", p=128))
            nc.sync.dma_start(xT_sb[:, 4:8, 0:512], xT_dr[:, 4:8, 0:512])
            nc.sync.dma_start(xT_sb[:, :, 512:1024], xT_dr[:, :, 512:1024])
            nc.sync.dma_start(
                wv_all.rearrange("p (dc ch) -> p dc ch", ch=CH),
                wv_d.rearrange("(dc p) ch -> p dc ch", p=128))
            nc.sync.dma_start(xT_sb[:, :, 1024:1536], xT_dr[:, :, 1024:1536])
            nc.sync.dma_start(xT_sb[:, :, 1536:2048], xT_dr[:, :, 1536:2048])
            nc.sync.dma_start(
                wo_all.rearrange("p (c d) -> p c d", d=D),
                wo_d.rearrange("(c p) d -> p c d", p=128))

            # dependency-free ACT warmup (loads the exp table early)
            warm = persist.tile([128, 1], F32, name="warm", tag="warm")
            nc.vector.memset(warm, 0.0)
            nc.scalar.activation(warm, warm, AF.Exp, bias=0.0, scale=1.0)

            # dependency-free PE warmup: fills the input-DMA head time and
            # establishes the >3us continuous-busy ramp so the first real
            # chains run at the full 2.4 GHz p-state
            pe0 = persist.tile([128, 512], BF16, name="pe0", tag="pe0")
            nc.vector.memset(pe0, 0.0)
            ps_w = ps_mm.tile([128, 512], F32, name="ps_warm", tag="ps_mm")
            for _ in range(8):
                nc.tensor.matmul(ps_w, lhsT=pe0[:, 0:128], rhs=pe0,
                                 start=True, stop=True)

            # bf16 identity for tensor-engine transposes in the endgame
            ident = persist.tile([128, 128], BF16, name="ident", tag="ident")
            nc.vector.memset(ident, 1.0)
            nc.gpsimd.affine_select(ident, ident, [[1, 128]],
                                    mybir.AluOpType.is_equal, 0.0,
                                    base=0, channel_multiplier=-1)

            qkT = [persist.tile([128, S], BF16, name=f"qkT{i}", tag=f"qkT{i}")
                   for i in range(4)]
            # token-major v, 4 heads x (64 dims + ones column)
            vsb = [persist.tile([128, HEADS_PER_CORE * 65], BF16,
                                name=f"v{i}", tag=f"v{i}") for i in range(NT)]
            # [d, S] layout consumed by out-proj (2 heads stacked per pair)
            vals = [persist.tile([128, S], BF16, name=f"vals{i}",
                                 tag=f"vals{i}") for i in range(2)]

            # --- helpers --------------------------------------------------
            qk_ps = {}

            def qk_part(ct, c, half):
                """Half (4 d-chunks) of a qk chain; bias-add on completion."""
                wsrc = wq_all if ct < 2 else wk_all
                wcol = (ct % 2) * 128
                if half == 0:
                    qk_ps[(ct, c)] = ps_mm.tile([128, 512], F32,
                                                name="ps_qk", tag="ps_mm")
                ps = qk_ps[(ct, c)]
                for dc in range(4 * half, 4 * half + 4):
                    nc.tensor.matmul(
                        ps,
                        lhsT=wsrc[:, CH * dc + wcol:CH * dc + wcol + 128],
                        rhs=xTs(dc, 512 * c, 512 * (c + 1)),
                        start=(dc == 0),
                        stop=(dc == ND - 1),
                    )
                if half == 1:
                    nc.vector.tensor_scalar_add(
                        qkT[ct][:, 512 * c:512 * (c + 1)], ps,
                        bias_all[:, ct:ct + 1])

            def qk_chain(ct, c):
                """qkT[ct][:, 512c:512c+512] = (W.T @ x) + bias."""
                qk_part(ct, c, 0)
                qk_part(ct, c, 1)

            def v_chain(t):
                """vsb[t]: token-major v for key tile t, ones col per head."""
                ps = ps_mm.tile([128, CH], F32, name="ps_v", tag="ps_mm")
                for dc in range(ND):
                    nc.tensor.matmul(
                        ps,
                        lhsT=xTs(dc, 128 * t, 128 * (t + 1)),
                        rhs=wv_all[:, CH * dc:CH * (dc + 1)],
                        start=(dc == 0),
                        stop=(dc == ND - 1),
                    )
                nc.vector.memset(vsb[t], 1.0)
                nc.vector.tensor_copy(
                    vsb[t].rearrange("p (h c) -> p h c", c=65)[:, :, 0:64],
                    ps.rearrange("p (h c) -> p h c", c=64),
                )

            def scores_exp(p, w, j):
                """One [128, 1024] psum tile: heads 2p|2p+1 scores for key
                tile j x query window w; one exp -> bf16 probs tile."""
                ps = ps_sc.tile([128, 1024], F32, name="ps_sc", tag="ps_sc")
                for hh in range(2):
                    nc.tensor.matmul(
                        ps[:, 512 * hh:512 * (hh + 1)],
                        lhsT=qkT[2 + p][64 * hh:64 * (hh + 1),
                                        128 * j:128 * (j + 1)],
                        rhs=qkT[p][64 * hh:64 * (hh + 1),
                                   512 * w:512 * (w + 1)],
                        start=True, stop=True,
                    )
                pr = probs_pool.tile([128, 1024], BF16, name="probs",
                                     tag="probs")
                nc.scalar.activation(pr, ps, AF.Exp, bias=0.0, scale=0.125)
                return pr

            probs_store = {}
            valsb_store = {}

            def val_mms(ps, p, hh, tq, probs_key, js, first, last):
                """Accumulate probs.T @ v over key tiles `js` into psum."""
                h = 2 * p + hh
                for j in js:
                    pr = probs_store[(probs_key, j)]
                    nc.tensor.matmul(
                        ps,
                        lhsT=pr[:, 512 * hh + 128 * tq:512 * hh + 128 * (tq + 1)],
                        rhs=vsb[j][:, 65 * h:65 * (h + 1)],
                        start=(first and j == js[0]),
                        stop=(last and j == js[-1]),
                        skip_group_check=not (first and last),
                    )

            pe_t_store = {}

            def val_drain(ps, p, w, hh, tq, on_act=False, pe_transpose=False):
                """Normalize psum -> valsb bf16; transpose when pair done."""
                if hh == 0:
                    vb = valsb_pool.tile([128, 128], BF16, name="valsb",
                                         tag="valsb")
                    valsb_store[(p, w, tq)] = vb
                else:
                    vb = valsb_store[(p, w, tq)]
                rc = rec_pool.tile([128, 1], F32, name="rec", tag="rec")
                nc.vector.reciprocal(rc, ps[:, 64:65])
                if on_act:
                    # ACT is idle post-exp: Copy with per-partition scale AP
                    nc.scalar.activation(vb[:, 64 * hh:64 * (hh + 1)],
                                         ps[:, 0:64], AF.Copy, bias=0.0,
                                         scale=rc)
                else:
                    nc.vector.tensor_scalar_mul(
                        vb[:, 64 * hh:64 * (hh + 1)], ps[:, 0:64], rc)
                if hh == 1:
                    if pe_transpose:
                        pe_t_store[tq] = (vb, p, w)
                    else:
                        # both heads packed: [128q, 128d] -> vals[p] via XBAR
                        nc.sync.dma_start(
                            vals[p][:, 512 * w + 128 * tq:
                                    512 * w + 128 * (tq + 1)],
                            vb, transpose=True)

            def pe_transpose_flush(tq, on_act=False):
                """Tensor-engine transpose (latency-critical endgame path)."""
                vb, p, w = pe_t_store.pop(tq)
                psT = ps_sc.tile([128, 128], BF16, name="ps_T", tag="ps_sc")
                nc.tensor.transpose(psT, vb, ident)
                dst = vals[p][:, 512 * w + 128 * tq:512 * w + 128 * (tq + 1)]
                if on_act:
                    nc.scalar.activation(dst, psT, AF.Copy, bias=0.0,
                                         scale=1.0)
                else:
                    nc.vector.tensor_copy(dst, psT)

            def val_chain(p, w, hh, tq, probs_key):
                """values[q, d] for head 2p+hh, q-tile tq of window w."""
                ps = ps_val.tile([128, 65], F32, name="ps_val", tag="ps_val")
                val_mms(ps, p, hh, tq, probs_key, list(range(NT)), True, True)
                val_drain(ps, p, w, hh, tq)

            outb_store = {}

            def outproj_half(t, mh, copy_act=False, pool=None, dma_sp=False):
                """One 512-col half of out tile t; DMA (via the idle Pool
                SWDGE so the SP queue stays clear for transposes) when both
                halves are done."""
                if mh == 0:
                    ob = outb_pool.tile([128, D], BF16, name="outb",
                                        tag="outb")
                    outb_store[t] = ob
                else:
                    ob = outb_store[t]
                if pool is None:
                    ps = ps_mm.tile([128, 512], F32, name="ps_out",
                                    tag="ps_mm")
                else:
                    ps = pool.tile([128, 512], F32, name="ps_out2",
                                   tag="ps_sc")
                for p in range(2):
                    nc.tensor.matmul(
                        ps,
                        lhsT=vals[p][:, 128 * t:128 * (t + 1)],
                        rhs=wo_all[:, D * p + 512 * mh:D * p + 512 * (mh + 1)],
                        start=(p == 0),
                        stop=(p == 1),
                    )
                if copy_act:
                    nc.scalar.activation(ob[:, 512 * mh:512 * (mh + 1)],
                                         ps, AF.Copy, bias=0.0, scale=1.0)
                else:
                    nc.vector.tensor_copy(ob[:, 512 * mh:512 * (mh + 1)], ps)
                if mh == 1:
                    if dma_sp:
                        nc.sync.dma_start(out_d[128 * t:128 * (t + 1), :], ob)
                    else:
                        nc.gpsimd.dma_start(out_d[128 * t:128 * (t + 1), :],
                                            ob)

            # --- schedule -------------------------------------------------
            windows = [(0, 0), (0, 1), (1, 0), (1, 1),
                       (0, 2), (0, 3), (1, 2), (1, 3)]
            # extra chain work per (window idx, j); qk chains split into two
            # 4-chunk halves on adjacent js so per-j PE load stays under the
            # ACT exp pace
            def _qk2(wi, j, ct, c):
                return {(wi, j): ("qkh", ct, c, 0), (wi, j + 1): ("qkh", ct, c, 1)}

            CHAINS = {
                (0, 4): ("v", 0), (0, 5): ("v", 1),
                (0, 8): ("v", 2), (0, 14): ("v", 3),
                (1, 0): ("v", 4), (1, 1): ("v", 5),
                (1, 8): ("v", 6), (1, 9): ("v", 7), (1, 10): ("v", 8),
                (1, 11): ("v", 9),
                (2, 2): ("v", 10), (2, 3): ("v", 11), (2, 6): ("v", 12),
                (2, 7): ("v", 13), (2, 10): ("v", 14), (2, 11): ("v", 15),
            }
            CHAINS.update(_qk2(0, 2, 2, 1))
            CHAINS.update(_qk2(0, 6, 2, 2))
            CHAINS.update(_qk2(0, 10, 2, 3))
            CHAINS.update(_qk2(0, 12, 0, 1))
            CHAINS.update(_qk2(1, 2, 1, 0))
            CHAINS.update(_qk2(1, 4, 3, 0))
            CHAINS.update(_qk2(1, 6, 3, 1))
            CHAINS.update(_qk2(2, 0, 3, 2))
            CHAINS.update(_qk2(2, 4, 3, 3))
            CHAINS.update(_qk2(2, 8, 1, 1))
            CHAINS.update(_qk2(3, 10, 0, 2))
            CHAINS.update(_qk2(4, 2, 0, 3))
            CHAINS.update(_qk2(5, 0, 1, 2))
            CHAINS.update(_qk2(6, 0, 1, 3))
            # values of window FRONT_VALUES[wi] run at js 0-7 of window wi
            FRONT_VALUES = {3: 0, 4: 1, 5: 4, 6: 5, 7: 6}
            # values of window BACK_VALUES[wi] run at js 8-15 of window wi
            BACK_VALUES = {3: 2, 4: 3}
            # out-proj token tiles per (window idx, j)
            # (window, j) -> (token tile, mh half); one half per j so the
            # per-j PE load stays under the ACT exp pace
            OUTPROJ = {}
            for _wi, _base in ((5, 0), (6, 4)):
                for _k in range(8):
                    OUTPROJ[(_wi, 8 + _k)] = (_base + _k // 2, _k % 2)
            OUTPROJ[(7, 10)] = (8, 0)
            OUTPROJ[(7, 11)] = (8, 1)
            OUTPROJ[(7, 13)] = (9, 0)
            OUTPROJ[(7, 14)] = (9, 1)

            def values_step(wi_src, jslot):
                """Chain #jslot (of 8) of window wi_src's values."""
                p, w = windows[wi_src]
                tq, hh = divmod(jslot, 2)
                val_chain(p, w, hh, tq, wi_src)

            # Last window's 8 values chains accumulate into paused psum
            # groups (4 chains packed per bank; only the first opens the
            # group, zeroing the whole bank's zero-region): js 0-6 in one
            # batch at j=8 (once ps_val is free of the front values), then
            # one key tile per j; the j=14,15 matmuls + drains run after the
            # final exp so only ~2 matmuls/chain trail the last score.
            def tail_part(tail_ps, jslot, js, first, last):
                bank, k = divmod(jslot, 4)
                ps = tail_ps[bank][:, 65 * k:65 * (k + 1)]
                tq, hh = divmod(jslot, 2)
                val_mms(ps, 1, hh, tq, 7, js, first, last)

            for _rep in range(reps):
                qk_chain(0, 0)
                qk_chain(2, 0)
                tail_ps = {}
                for wi, (p, w) in enumerate(windows):
                    for j in range(NT):
                        extra = CHAINS.get((wi, j))
                        if extra is not None:
                            if extra[0] == "qkh":
                                qk_part(extra[1], extra[2], extra[3])
                            else:
                                v_chain(extra[1])
                        if wi in FRONT_VALUES and j < 8:
                            values_step(FRONT_VALUES[wi], j)
                        if wi in BACK_VALUES and j >= 8:
                            values_step(BACK_VALUES[wi], j - 8)
                        probs_store[(wi, j)] = scores_exp(p, w, j)
                        oph = OUTPROJ.get((wi, j))
                        if oph is not None:
                            outproj_half(*oph)
                        if wi == 7 and j in (8, 9):
                            bank = j - 8
                            tail_ps[bank] = ps_val.tile(
                                [128, 260], F32, name="ps_tail",
                                tag="ps_val")
                            for jslot in range(4 * bank, 4 * bank + 4):
                                tail_part(tail_ps, jslot, list(range(7)),
                                          jslot % 4 == 0, False)
                        elif wi == 7 and j == 10:
                            for jslot in range(8):
                                tail_part(tail_ps, jslot, [7, 8], False,
                                          False)
                        elif wi == 7 and j >= 11:
                            for jslot in range(8):
                                tail_part(tail_ps, jslot, [j - 2], False,
                                          False)
                # tail: last two key tiles of each paused chain; drains split
                # ACT/DVE; transposes on the tensor engine (no DMA latency),
                # out-proj 10/11 matmuls fill the PE between them
                for jslot in range(7, -1, -1):
                    tail_part(tail_ps, jslot, [14, 15], False, True)
                for tq in range(3, -1, -1):
                    for hh in range(2):
                        bank, k = divmod(2 * tq + hh, 4)
                        val_drain(tail_ps[bank][:, 65 * k:65 * (k + 1)],
                                  1, 3, hh, tq, on_act=(hh == 0),
                                  pe_transpose=True)
                outproj_half(10, 0)
                pe_transpose_flush(3)
                outproj_half(10, 1)
                pe_transpose_flush(2, on_act=True)
                outproj_half(11, 0, copy_act=True)
                pe_transpose_flush(1)
                outproj_half(11, 1, copy_act=True)
                pe_transpose_flush(0, on_act=True)
                for t, ca in ((15, True), (14, False), (13, True),
                              (12, False)):
                    pool = ps_sc if t in (15, 13) else None
                    outproj_half(t, 0, copy_act=ca, pool=pool, dma_sp=True)
                    outproj_half(t, 1, copy_act=ca, pool=pool, dma_sp=True)
                # drop references so the next rep re-allocates cleanly
                probs_store.clear()
                valsb_store.clear()
                # drop references so the next rep re-allocates cleanly
                probs_store.clear()
                valsb_store.clear()

    nc.compile()
    return nc


def make_in_maps(x, W_qkv, b_qkv, W_out, n_cores=8):
    """Per-core input dict: core c -> batch c//4, head group c%4."""
    bf = ml_dtypes.bfloat16
    in_maps = []
    for c in range(n_cores):
        b, g = divmod(c, 4)
        heads = range(HEADS_PER_CORE * g, HEADS_PER_CORE * (g + 1))
        qs = np.concatenate([W_qkv[:, 192 * h:192 * h + 64] for h in heads], 1)
        ks = np.concatenate([W_qkv[:, 192 * h + 64:192 * h + 128] for h in heads], 1)
        vs = np.concatenate([W_qkv[:, 192 * h + 128:192 * h + 192] for h in heads], 1)
        bq = np.concatenate([b_qkv[192 * h:192 * h + 64] for h in heads])
        bk = np.concatenate([b_qkv[192 * h + 64:192 * h + 128] for h in heads])
        in_maps.append({
            "xT": np.ascontiguousarray(x[b].T).astype(bf),
            "wq": np.ascontiguousarray(qs).astype(bf),
            "wk": np.ascontiguousarray(ks).astype(bf),
            "wv": np.ascontiguousarray(vs).astype(bf),
            "wo": np.ascontiguousarray(W_out[CH * g:CH * (g + 1)]).astype(bf),
            "bqk": np.stack([bq[:128], bq[128:], bk[:128], bk[128:]])
                     .reshape(4, 128, 1).astype(np.float32),
        })
    return in_maps


_PROGRAM_CACHE = {}


def _get_program(S):
    if S not in _PROGRAM_CACHE:
        _PROGRAM_CACHE[S] = build_core_program(S=S)
    return _PROGRAM_CACHE[S]


class PjrtRunner:
    """Reusable compiled SPMD executable (no donation, so it can be re-run
    back-to-back on device-resident inputs for timing)."""

    def __init__(self, nc, n_cores=8):
        import jax
        from jax.sharding import Mesh, PartitionSpec
        from jax.experimental.shard_map import shard_map
        from concourse import bass2jax, mybir as mb

        bass2jax.install_neuronx_cc_hook()
        self.nc = nc
        self.n_cores = n_cores
        in_names, out_names, out_avals, zero_outs = [], [], [], []
        for alloc in nc.m.functions[0].allocations:
            if not isinstance(alloc, mb.MemoryLocationSet):
                continue
            name = alloc.memorylocations[0].name
            if alloc.kind == "ExternalInput":
                in_names.append(name)
            elif alloc.kind == "ExternalOutput":
                out_names.append(name)
                shape = tuple(alloc.tensor_shape)
                dtype = mb.dt.np(alloc.dtype)
                out_avals.append(jax.core.ShapedArray(shape, dtype))
                zero_outs.append(np.zeros(shape, dtype))
        self.in_names = list(in_names)
        self.out_names = out_names
        self.out_avals = out_avals
        self.zero_outs = zero_outs
        n_params = len(in_names)
        all_names = in_names + out_names

        def _body(*args):
            outs = bass2jax._bass_exec_p.bind(
                *args,
                out_avals=tuple(out_avals),
                in_names=tuple(all_names),
                out_names=tuple(out_names),
                lowering_input_output_aliases=(),
                sim_require_finite=True,
                sim_require_nnan=True,
                nc=nc,
            )
            return tuple(outs)

        devices = jax.devices()[:n_cores]
        self.mesh = Mesh(np.asarray(devices), ("core",))
        in_specs = (PartitionSpec("core"),) * (n_params + len(out_names))
        out_specs = (PartitionSpec("core"),) * len(out_names)
        self.fn = jax.jit(
            shard_map(_body, mesh=self.mesh, in_specs=in_specs,
                      out_specs=out_specs, check_rep=False),
            keep_unused=True,
        )
        self._dev_args = None

    def stage(self, in_maps):
        """Concatenate per-core inputs, upload once, keep device arrays."""
        import jax
        from jax.sharding import NamedSharding, PartitionSpec
        n = self.n_cores
        concat = [
            np.concatenate([np.asarray(in_maps[c][k]) for c in range(n)], axis=0)
            for k in self.in_names
        ]
        concat += [
            np.zeros((n * z.shape[0], *z.shape[1:]), z.dtype)
            for z in self.zero_outs
        ]
        sh = NamedSharding(self.mesh, PartitionSpec("core"))
        self._dev_args = [jax.device_put(a, sh) for a in concat]

    def run(self):
        outs = self.fn(*self._dev_args)
        # keep device arrays for reuse; pull results to host
        res = []
        for c in range(self.n_cores):
            res.append({
                name: np.asarray(outs[i]).reshape(
                    self.n_cores, *self.out_avals[i].shape)[c]
                for i, name in enumerate(self.out_names)
            })
        return res

    def time_iters(self, iters=20):
        import time
        import jax
        outs = self.fn(*self._dev_args)
        jax.block_until_ready(outs)
        t0 = time.perf_counter()
        for _ in range(iters):
            outs = self.fn(*self._dev_args)
        jax.block_until_ready(outs)
        t1 = time.perf_counter()
        return (t1 - t0) / iters


_RUNNER_CACHE = {}


def get_runner(S):
    if S not in _RUNNER_CACHE:
        _RUNNER_CACHE[S] = PjrtRunner(_get_program(S))
    return _RUNNER_CACHE[S]


def combine_outputs(results, W_qkv, b_qkv, W_out, b_out, B, S, D):
    b_v = np.concatenate([b_qkv[192 * h + 128:192 * h + 192] for h in range(16)])
    corr = (b_v.astype(np.float64) @ W_out.astype(np.float64)).astype(np.float32)
    corr += b_out
    out = np.zeros((B, S, D), np.float32)
    for c in range(8):
        out[c // 4] += results[c]["out"].astype(np.float32)
    out += corr[None, None, :]
    return out


def kernel(x, W_qkv, b_qkv, W_out, b_out):
    x = np.asarray(x)
    W_qkv = np.asarray(W_qkv)
    b_qkv = np.asarray(b_qkv)
    W_out = np.asarray(W_out)
    b_out = np.asarray(b_out)
    B, S, D = x.shape

    runner = get_runner(S)
    runner.stage(make_in_maps(x, W_qkv, b_qkv, W_out))
    results = runner.run()
    return combine_outputs(results, W_qkv, b_qkv, W_out, b_out, B, S, D)


# revision 49
# speedup vs baseline: 1.6886x; 1.2490x over previous
"""Multi-head self-attention kernel for Trainium2 (8 NeuronCores, Bass/Tile).

Sharding: 8 cores = 2 batches x 4 head-groups; each core computes one batch
and 4 heads end to end (no collectives); host sums the 4 bf16 partial
out-projections per batch and folds in the v-bias + out-bias correction.

Per-core dataflow (v2):
  - qkT[ct] = (W.T @ x) + bias, ct in {q01,q23,k01,k24}, [128, S] each
    (two heads stacked on partitions 0:64 / 64:128).
  - scores for (pair p, 512-query window w, key tile j): ONE [128, 1024]
    PSUM tile per j holding both heads side by side; ONE exp per j on ACT.
  - values flipped vs v1: out[q, d] with probs as stationary (N=65 moving
    cols incl a ones column that yields the softmax denominator), PSUM-
    accumulated over all 16 key tiles -> half the tensor-engine rows.
  - normalize on DVE via per-partition reciprocal + tensor_scalar_mul,
    pack two heads into a [128, 128] tile, DMA-XBAR transpose into the
    [d, S] vals layout consumed by the out-projection.
  - out-projection: 16 token tiles, 2x2 accumulated matmuls each, DVE
    copies, DMA out.
Window order: (0,0),(0,1),(1,0),(1,1),(0,2),(0,3),(1,2),(1,3); values of
window i run during window i+3/i+2 (front) or i+1 (back) to keep the PE
fed at the ACT exp pace; out-projections trail once both pairs of a
query window have been transposed.
"""


import numpy as np
import ml_dtypes
import sys

try:
    import concourse.bass as bass
except ImportError:  # pragma: no cover
    sys.path.insert(0, "/opt/trn_rl_repo")
    import concourse.bass as bass

import concourse.bacc as bacc
import concourse.mybir as mybir
import concourse.tile as tile
from concourse.bass_utils import run_bass_kernel_spmd

BF16 = mybir.dt.bfloat16
F32 = mybir.dt.float32
AF = mybir.ActivationFunctionType

D_MODEL = 1024
HEADS_PER_CORE = 4
HEAD_DIM = 64
CH = HEADS_PER_CORE * HEAD_DIM  # 256


def build_core_program(S=2048, D=D_MODEL, reps=1):
    nc = bacc.Bacc(trn_type="TRN2", target_bir_lowering=False, debug=False,
                   enable_partition_id=False)

    xT_d = nc.dram_tensor("xT", [D, S], BF16, kind="ExternalInput").ap()
    wq_d = nc.dram_tensor("wq", [D, CH], BF16, kind="ExternalInput").ap()
    wk_d = nc.dram_tensor("wk", [D, CH], BF16, kind="ExternalInput").ap()
    wv_d = nc.dram_tensor("wv", [D, CH], BF16, kind="ExternalInput").ap()
    wo_d = nc.dram_tensor("wo", [CH, D], BF16, kind="ExternalInput").ap()
    bqk_d = nc.dram_tensor("bqk", [4, 128, 1], F32, kind="ExternalInput").ap()
    out_d = nc.dram_tensor("out", [S, D], BF16, kind="ExternalOutput").ap()

    NT = S // 128     # key tiles
    ND = D // 128     # d_model contraction chunks
    NQ = S // 512     # query windows per pair
    assert NT == 16 and NQ == 4 and ND == 8

    with tile.TileContext(nc) as tc:
        with (
            tc.tile_pool(name="persist", bufs=1) as persist,
            tc.tile_pool(name="probs", bufs=56) as probs_pool,
            tc.tile_pool(name="valsb", bufs=6) as valsb_pool,
            tc.tile_pool(name="rec", bufs=6) as rec_pool,
            tc.tile_pool(name="outb", bufs=2) as outb_pool,
            tc.tile_pool(name="ps_sc", bufs=2, space="PSUM") as ps_sc,
            tc.tile_pool(name="ps_mm", bufs=2, space="PSUM") as ps_mm,
            tc.tile_pool(name="ps_val", bufs=2, space="PSUM") as ps_val,
        ):
            # --- constants ------------------------------------------------
            # consolidated SBUF images: one strided DMA each instead of
            # per-chunk DMAs (HWDGE trigger overhead dominates small DMAs)
            xT_all = persist.tile([128, ND * S], BF16, name="xT", tag="xT")
            wq_all = persist.tile([128, ND * CH], BF16, name="wq", tag="wq")
            wk_all = persist.tile([128, ND * CH], BF16, name="wk", tag="wk")
            wv_all = persist.tile([128, ND * CH], BF16, name="wv", tag="wv")
            bias_all = persist.tile([128, 4], F32, name="bias", tag="bias")
            wo_all = persist.tile([128, 2 * D], BF16, name="wo", tag="wo")

            def xTs(dc, c0, c1):
                return xT_all[:, S * dc + c0:S * dc + c1]

            xT_dr = xT_d.rearrange("(dc p) s -> p dc s", p=128)
            xT_sb = xT_all.rearrange("p (dc s) -> p dc s", s=S)
            # priority order: pieces needed by the first qk chains first;
            # all triggers on SP so the ACT sequencer stays free for exps
            wq_sb = wq_all.rearrange("p (dc ch) -> p dc ch", ch=CH)
            wq_dr = wq_d.rearrange("(dc p) ch -> p dc ch", p=128)
            wk_sb = wk_all.rearrange("p (dc ch) -> p dc ch", ch=CH)
            wk_dr = wk_d.rearrange("(dc p) ch -> p dc ch", p=128)
            # critical path to the first exp: wq -> xT cols 0:512 -> wk;
            # the first k chain is narrowed to key tokens 0:128 so it only
            # trails wk by ~0.5us
            nc.sync.dma_start(
                bias_all.rearrange("p b -> p b ()"),
                bqk_d.rearrange("b p one -> p b one"))
            nc.sync.dma_start(wq_sb, wq_dr)
            nc.sync.dma_start(xT_sb[:, 0:4, 0:512], xT_dr[:, 0:4, 0:512])
            nc.sync.dma_start(xT_sb[:, 4:8, 0:512], xT_dr[:, 4:8, 0:512])
            nc.sync.dma_start(wk_sb, wk_dr)
            nc.sync.dma_start(xT_sb[:, :, 512:1024], xT_dr[:, :, 512:1024])
            nc.sync.dma_start(
                wv_all.rearrange("p (dc ch) -> p dc ch", ch=CH),
                wv_d.rearrange("(dc p) ch -> p dc ch", p=128))
            nc.sync.dma_start(xT_sb[:, :, 1024:1536], xT_dr[:, :, 1024:1536])
            nc.sync.dma_start(xT_sb[:, :, 1536:2048], xT_dr[:, :, 1536:2048])
            nc.sync.dma_start(
                wo_all.rearrange("p (c d) -> p c d", d=D),
                wo_d.rearrange("(c p) d -> p c d", p=128))

            # dependency-free ACT warmup (loads the exp table early)
            warm = persist.tile([128, 1], F32, name="warm", tag="warm")
            nc.vector.memset(warm, 0.0)
            nc.scalar.activation(warm, warm, AF.Exp, bias=0.0, scale=1.0)

            # dependency-free PE warmup: fills the input-DMA head time and
            # establishes the >3us continuous-busy ramp so the first real
            # chains run at the full 2.4 GHz p-state
            pe0 = persist.tile([128, 512], BF16, name="pe0", tag="pe0")
            nc.vector.memset(pe0, 0.0)
            ps_w = ps_mm.tile([128, 512], F32, name="ps_warm", tag="ps_mm")
            for _ in range(6):
                nc.tensor.matmul(ps_w, lhsT=pe0[:, 0:128], rhs=pe0,
                                 start=True, stop=True)

            # bf16 identity for tensor-engine transposes in the endgame
            ident = persist.tile([128, 128], BF16, name="ident", tag="ident")
            nc.vector.memset(ident, 1.0)
            nc.gpsimd.affine_select(ident, ident, [[1, 128]],
                                    mybir.AluOpType.is_equal, 0.0,
                                    base=0, channel_multiplier=-1)

            qkT = [persist.tile([128, S], BF16, name=f"qkT{i}", tag=f"qkT{i}")
                   for i in range(4)]
            # token-major v, 4 heads x (64 dims + ones column)
            vsb = [persist.tile([128, HEADS_PER_CORE * 65], BF16,
                                name=f"v{i}", tag=f"v{i}") for i in range(NT)]
            # [d, S] layout consumed by out-proj (2 heads stacked per pair)
            vals = [persist.tile([128, S], BF16, name=f"vals{i}",
                                 tag=f"vals{i}") for i in range(2)]

            # --- helpers --------------------------------------------------
            qk_ps = {}

            def qk_part(ct, c, half):
                """Half (4 d-chunks) of a qk chain; bias-add on completion."""
                wsrc = wq_all if ct < 2 else wk_all
                wcol = (ct % 2) * 128
                if half == 0:
                    qk_ps[(ct, c)] = ps_mm.tile([128, 512], F32,
                                                name="ps_qk", tag="ps_mm")
                ps = qk_ps[(ct, c)]
                for dc in range(4 * half, 4 * half + 4):
                    nc.tensor.matmul(
                        ps,
                        lhsT=wsrc[:, CH * dc + wcol:CH * dc + wcol + 128],
                        rhs=xTs(dc, 512 * c, 512 * (c + 1)),
                        start=(dc == 0),
                        stop=(dc == ND - 1),
                    )
                if half == 1:
                    nc.vector.tensor_scalar_add(
                        qkT[ct][:, 512 * c:512 * (c + 1)], ps,
                        bias_all[:, ct:ct + 1])

            def qk_chain(ct, c):
                """qkT[ct][:, 512c:512c+512] = (W.T @ x) + bias."""
                qk_part(ct, c, 0)
                qk_part(ct, c, 1)

            def qk_narrow(ct, c0, c1):
                """qkT[ct][:, c0:c1] only — for the latency-critical head."""
                wsrc = wq_all if ct < 2 else wk_all
                wcol = (ct % 2) * 128
                ps = ps_mm.tile([128, c1 - c0], F32, name="ps_qkn",
                                tag="ps_mm")
                for dc in range(ND):
                    nc.tensor.matmul(
                        ps,
                        lhsT=wsrc[:, CH * dc + wcol:CH * dc + wcol + 128],
                        rhs=xTs(dc, c0, c1),
                        start=(dc == 0),
                        stop=(dc == ND - 1),
                    )
                nc.vector.tensor_scalar_add(
                    qkT[ct][:, c0:c1], ps, bias_all[:, ct:ct + 1])

            def v_chain(t):
                """vsb[t]: token-major v for key tile t, ones col per head."""
                ps = ps_mm.tile([128, CH], F32, name="ps_v", tag="ps_mm")
                for dc in range(ND):
                    nc.tensor.matmul(
                        ps,
                        lhsT=xTs(dc, 128 * t, 128 * (t + 1)),
                        rhs=wv_all[:, CH * dc:CH * (dc + 1)],
                        start=(dc == 0),
                        stop=(dc == ND - 1),
                    )
                nc.vector.memset(vsb[t], 1.0)
                nc.vector.tensor_copy(
                    vsb[t].rearrange("p (h c) -> p h c", c=65)[:, :, 0:64],
                    ps.rearrange("p (h c) -> p h c", c=64),
                )

            def scores_exp(p, w, j):
                """One [128, 1024] psum tile: heads 2p|2p+1 scores for key
                tile j x query window w; one exp -> bf16 probs tile."""
                ps = ps_sc.tile([128, 1024], F32, name="ps_sc", tag="ps_sc")
                for hh in range(2):
                    nc.tensor.matmul(
                        ps[:, 512 * hh:512 * (hh + 1)],
                        lhsT=qkT[2 + p][64 * hh:64 * (hh + 1),
                                        128 * j:128 * (j + 1)],
                        rhs=qkT[p][64 * hh:64 * (hh + 1),
                                   512 * w:512 * (w + 1)],
                        start=True, stop=True,
                    )
                pr = probs_pool.tile([128, 1024], BF16, name="probs",
                                     tag="probs")
                nc.scalar.activation(pr, ps, AF.Exp, bias=0.0, scale=0.125)
                return pr

            probs_store = {}
            valsb_store = {}

            def val_mms(ps, p, hh, tq, probs_key, js, first, last):
                """Accumulate probs.T @ v over key tiles `js` into psum."""
                h = 2 * p + hh
                for j in js:
                    pr = probs_store[(probs_key, j)]
                    nc.tensor.matmul(
                        ps,
                        lhsT=pr[:, 512 * hh + 128 * tq:512 * hh + 128 * (tq + 1)],
                        rhs=vsb[j][:, 65 * h:65 * (h + 1)],
                        start=(first and j == js[0]),
                        stop=(last and j == js[-1]),
                        skip_group_check=not (first and last),
                    )

            pe_t_store = {}

            def val_drain(ps, p, w, hh, tq, on_act=False, pe_transpose=False):
                """Normalize psum -> valsb bf16; transpose when pair done."""
                if hh == 0:
                    vb = valsb_pool.tile([128, 128], BF16, name="valsb",
                                         tag="valsb")
                    valsb_store[(p, w, tq)] = vb
                else:
                    vb = valsb_store[(p, w, tq)]
                rc = rec_pool.tile([128, 1], F32, name="rec", tag="rec")
                nc.vector.reciprocal(rc, ps[:, 64:65])
                if on_act:
                    # ACT is idle post-exp: Copy with per-partition scale AP
                    nc.scalar.activation(vb[:, 64 * hh:64 * (hh + 1)],
                                         ps[:, 0:64], AF.Copy, bias=0.0,
                                         scale=rc)
                else:
                    nc.vector.tensor_scalar_mul(
                        vb[:, 64 * hh:64 * (hh + 1)], ps[:, 0:64], rc)
                if hh == 1:
                    if pe_transpose:
                        pe_t_store[tq] = (vb, p, w)
                    else:
                        # both heads packed: [128q, 128d] -> vals[p] via XBAR
                        nc.sync.dma_start(
                            vals[p][:, 512 * w + 128 * tq:
                                    512 * w + 128 * (tq + 1)],
                            vb, transpose=True)

            def pe_transpose_flush(tq, on_act=False):
                """Tensor-engine transpose (latency-critical endgame path)."""
                vb, p, w = pe_t_store.pop(tq)
                psT = ps_sc.tile([128, 128], BF16, name="ps_T", tag="ps_sc")
                nc.tensor.transpose(psT, vb, ident)
                dst = vals[p][:, 512 * w + 128 * tq:512 * w + 128 * (tq + 1)]
                if on_act:
                    nc.scalar.activation(dst, psT, AF.Copy, bias=0.0,
                                         scale=1.0)
                else:
                    nc.vector.tensor_copy(dst, psT)

            def val_chain(p, w, hh, tq, probs_key):
                """values[q, d] for head 2p+hh, q-tile tq of window w."""
                ps = ps_val.tile([128, 65], F32, name="ps_val", tag="ps_val")
                val_mms(ps, p, hh, tq, probs_key, list(range(NT)), True, True)
                val_drain(ps, p, w, hh, tq)

            outb_store = {}

            def outproj_half(t, mh, copy_act=False, pool=None, dma_sp=False):
                """One 512-col half of out tile t; DMA (via the idle Pool
                SWDGE so the SP queue stays clear for transposes) when both
                halves are done."""
                if mh == 0:
                    ob = outb_pool.tile([128, D], BF16, name="outb",
                                        tag="outb")
                    outb_store[t] = ob
                else:
                    ob = outb_store[t]
                if pool is None:
                    ps = ps_mm.tile([128, 512], F32, name="ps_out",
                                    tag="ps_mm")
                else:
                    ps = pool.tile([128, 512], F32, name="ps_out2",
                                   tag="ps_sc")
                for p in range(2):
                    nc.tensor.matmul(
                        ps,
                        lhsT=vals[p][:, 128 * t:128 * (t + 1)],
                        rhs=wo_all[:, D * p + 512 * mh:D * p + 512 * (mh + 1)],
                        start=(p == 0),
                        stop=(p == 1),
                    )
                if copy_act:
                    nc.scalar.activation(ob[:, 512 * mh:512 * (mh + 1)],
                                         ps, AF.Copy, bias=0.0, scale=1.0)
                else:
                    nc.vector.tensor_copy(ob[:, 512 * mh:512 * (mh + 1)], ps)
                if mh == 1:
                    if dma_sp:
                        nc.sync.dma_start(out_d[128 * t:128 * (t + 1), :], ob)
                    else:
                        nc.gpsimd.dma_start(out_d[128 * t:128 * (t + 1), :],
                                            ob)

            # --- schedule -------------------------------------------------
            windows = [(0, 0), (0, 1), (1, 0), (1, 1),
                       (0, 2), (0, 3), (1, 2), (1, 3)]
            # extra chain work per (window idx, j); qk chains split into two
            # 4-chunk halves on adjacent js so per-j PE load stays under the
            # ACT exp pace
            def _qk2(wi, j, ct, c):
                return {(wi, j): ("qkh", ct, c, 0), (wi, j + 1): ("qkh", ct, c, 1)}

            CHAINS = {
                (0, 4): ("v", 0), (0, 5): ("v", 1),
                (0, 8): ("v", 2), (0, 14): ("v", 3),
                (1, 0): ("v", 4), (1, 1): ("v", 5),
                (1, 8): ("v", 6), (1, 9): ("v", 7), (1, 10): ("v", 8),
                (1, 11): ("v", 9),
                (2, 2): ("v", 10), (2, 3): ("v", 11), (2, 6): ("v", 12),
                (2, 7): ("v", 13), (2, 10): ("v", 14), (2, 11): ("v", 15),
            }
            CHAINS.update(_qk2(0, 2, 2, 1))
            CHAINS.update(_qk2(0, 6, 2, 2))
            CHAINS.update(_qk2(0, 10, 2, 3))
            CHAINS.update(_qk2(0, 12, 0, 1))
            CHAINS.update(_qk2(1, 2, 1, 0))
            CHAINS.update(_qk2(1, 4, 3, 0))
            CHAINS.update(_qk2(1, 6, 3, 1))
            CHAINS.update(_qk2(2, 0, 3, 2))
            CHAINS.update(_qk2(2, 4, 3, 3))
            CHAINS.update(_qk2(2, 8, 1, 1))
            CHAINS.update(_qk2(3, 10, 0, 2))
            CHAINS.update(_qk2(4, 2, 0, 3))
            CHAINS.update(_qk2(5, 0, 1, 2))
            CHAINS.update(_qk2(6, 0, 1, 3))
            # values of window FRONT_VALUES[wi] run at js 0-7 of window wi
            FRONT_VALUES = {3: 0, 4: 1, 5: 4, 6: 5, 7: 6}
            # values of window BACK_VALUES[wi] run at js 8-15 of window wi
            BACK_VALUES = {3: 2, 4: 3}
            # out-proj token tiles per (window idx, j)
            # (window, j) -> (token tile, mh half); one half per j so the
            # per-j PE load stays under the ACT exp pace
            OUTPROJ = {}
            for _wi, _base in ((5, 0), (6, 4)):
                for _k in range(8):
                    OUTPROJ[(_wi, 8 + _k)] = (_base + _k // 2, _k % 2)
            OUTPROJ[(7, 10)] = (8, 0)
            OUTPROJ[(7, 11)] = (8, 1)
            OUTPROJ[(7, 13)] = (9, 0)
            OUTPROJ[(7, 14)] = (9, 1)

            def values_step(wi_src, jslot):
                """Chain #jslot (of 8) of window wi_src's values."""
                p, w = windows[wi_src]
                tq, hh = divmod(jslot, 2)
                val_chain(p, w, hh, tq, wi_src)

            # Last window's 8 values chains accumulate into paused psum
            # groups (4 chains packed per bank; only the first opens the
            # group, zeroing the whole bank's zero-region): js 0-6 in one
            # batch at j=8 (once ps_val is free of the front values), then
            # one key tile per j; the j=14,15 matmuls + drains run after the
            # final exp so only ~2 matmuls/chain trail the last score.
            def tail_part(tail_ps, jslot, js, first, last):
                bank, k = divmod(jslot, 4)
                ps = tail_ps[bank][:, 65 * k:65 * (k + 1)]
                tq, hh = divmod(jslot, 2)
                val_mms(ps, 1, hh, tq, 7, js, first, last)

            for _rep in range(reps):
                qk_chain(0, 0)
                qk_narrow(2, 0, 128)
                qk_narrow(2, 128, 512)
                tail_ps = {}
                for wi, (p, w) in enumerate(windows):
                    for j in range(NT):
                        extra = CHAINS.get((wi, j))
                        if extra is not None:
                            if extra[0] == "qkh":
                                qk_part(extra[1], extra[2], extra[3])
                            else:
                                v_chain(extra[1])
                        if wi in FRONT_VALUES and j < 8:
                            values_step(FRONT_VALUES[wi], j)
                        if wi in BACK_VALUES and j >= 8:
                            values_step(BACK_VALUES[wi], j - 8)
                        probs_store[(wi, j)] = scores_exp(p, w, j)
                        oph = OUTPROJ.get((wi, j))
                        if oph is not None:
                            outproj_half(*oph)
                        if wi == 7 and j in (8, 9):
                            bank = j - 8
                            tail_ps[bank] = ps_val.tile(
                                [128, 260], F32, name="ps_tail",
                                tag="ps_val")
                            for jslot in range(4 * bank, 4 * bank + 4):
                                tail_part(tail_ps, jslot, list(range(7)),
                                          jslot % 4 == 0, False)
                        elif wi == 7 and j == 10:
                            for jslot in range(8):
                                tail_part(tail_ps, jslot, [7, 8], False,
                                          False)
                        elif wi == 7 and j >= 11:
                            for jslot in range(8):
                                tail_part(tail_ps, jslot, [j - 2], False,
                                          False)
                # tail: last two key tiles of each paused chain; drains split
                # ACT/DVE; transposes on the tensor engine (no DMA latency),
                # out-proj 10/11 matmuls fill the PE between them
                for jslot in range(7, -1, -1):
                    tail_part(tail_ps, jslot, [14, 15], False, True)
                for tq in range(3, -1, -1):
                    for hh in range(2):
                        bank, k = divmod(2 * tq + hh, 4)
                        val_drain(tail_ps[bank][:, 65 * k:65 * (k + 1)],
                                  1, 3, hh, tq, on_act=(hh == 0),
                                  pe_transpose=True)
                outproj_half(10, 0)
                pe_transpose_flush(3)
                outproj_half(10, 1)
                pe_transpose_flush(2, on_act=True)
                outproj_half(11, 0, copy_act=True)
                pe_transpose_flush(1)
                outproj_half(11, 1, copy_act=True)
                pe_transpose_flush(0, on_act=True)
                for t in (15, 14, 13, 12):
                    pool = ps_sc if t in (15, 13) else None
                    # alternate copy engines so the two halves land in
                    # parallel on ACT and DVE; spread the final DMAs over
                    # the SP and Pool queues so they don't serialize
                    outproj_half(t, 0, copy_act=True, pool=pool, dma_sp=True)
                    outproj_half(t, 1, copy_act=False, pool=pool,
                                 dma_sp=True)
                # drop references so the next rep re-allocates cleanly
                probs_store.clear()
                valsb_store.clear()
                # drop references so the next rep re-allocates cleanly
                probs_store.clear()
                valsb_store.clear()

    nc.compile()
    return nc


def make_in_maps(x, W_qkv, b_qkv, W_out, n_cores=8):
    """Per-core input dict: core c -> batch c//4, head group c%4."""
    bf = ml_dtypes.bfloat16
    in_maps = []
    for c in range(n_cores):
        b, g = divmod(c, 4)
        heads = range(HEADS_PER_CORE * g, HEADS_PER_CORE * (g + 1))
        qs = np.concatenate([W_qkv[:, 192 * h:192 * h + 64] for h in heads], 1)
        ks = np.concatenate([W_qkv[:, 192 * h + 64:192 * h + 128] for h in heads], 1)
        vs = np.concatenate([W_qkv[:, 192 * h + 128:192 * h + 192] for h in heads], 1)
        bq = np.concatenate([b_qkv[192 * h:192 * h + 64] for h in heads])
        bk = np.concatenate([b_qkv[192 * h + 64:192 * h + 128] for h in heads])
        in_maps.append({
            "xT": np.ascontiguousarray(x[b].T).astype(bf),
            "wq": np.ascontiguousarray(qs).astype(bf),
            "wk": np.ascontiguousarray(ks).astype(bf),
            "wv": np.ascontiguousarray(vs).astype(bf),
            "wo": np.ascontiguousarray(W_out[CH * g:CH * (g + 1)]).astype(bf),
            "bqk": np.stack([bq[:128], bq[128:], bk[:128], bk[128:]])
                     .reshape(4, 128, 1).astype(np.float32),
        })
    return in_maps


_PROGRAM_CACHE = {}


def _get_program(S):
    if S not in _PROGRAM_CACHE:
        _PROGRAM_CACHE[S] = build_core_program(S=S)
    return _PROGRAM_CACHE[S]


class PjrtRunner:
    """Reusable compiled SPMD executable (no donation, so it can be re-run
    back-to-back on device-resident inputs for timing)."""

    def __init__(self, nc, n_cores=8):
        import jax
        from jax.sharding import Mesh, PartitionSpec
        from jax.experimental.shard_map import shard_map
        from concourse import bass2jax, mybir as mb

        bass2jax.install_neuronx_cc_hook()
        self.nc = nc
        self.n_cores = n_cores
        in_names, out_names, out_avals, zero_outs = [], [], [], []
        for alloc in nc.m.functions[0].allocations:
            if not isinstance(alloc, mb.MemoryLocationSet):
                continue
            name = alloc.memorylocations[0].name
            if alloc.kind == "ExternalInput":
                in_names.append(name)
            elif alloc.kind == "ExternalOutput":
                out_names.append(name)
                shape = tuple(alloc.tensor_shape)
                dtype = mb.dt.np(alloc.dtype)
                out_avals.append(jax.core.ShapedArray(shape, dtype))
                zero_outs.append(np.zeros(shape, dtype))
        self.in_names = list(in_names)
        self.out_names = out_names
        self.out_avals = out_avals
        self.zero_outs = zero_outs
        n_params = len(in_names)
        all_names = in_names + out_names

        def _body(*args):
            outs = bass2jax._bass_exec_p.bind(
                *args,
                out_avals=tuple(out_avals),
                in_names=tuple(all_names),
                out_names=tuple(out_names),
                lowering_input_output_aliases=(),
                sim_require_finite=True,
                sim_require_nnan=True,
                nc=nc,
            )
            return tuple(outs)

        devices = jax.devices()[:n_cores]
        self.mesh = Mesh(np.asarray(devices), ("core",))
        in_specs = (PartitionSpec("core"),) * (n_params + len(out_names))
        out_specs = (PartitionSpec("core"),) * len(out_names)
        self.fn = jax.jit(
            shard_map(_body, mesh=self.mesh, in_specs=in_specs,
                      out_specs=out_specs, check_rep=False),
            keep_unused=True,
        )
        self._dev_args = None

    def stage(self, in_maps):
        """Concatenate per-core inputs, upload once, keep device arrays."""
        import jax
        from jax.sharding import NamedSharding, PartitionSpec
        n = self.n_cores
        concat = [
            np.concatenate([np.asarray(in_maps[c][k]) for c in range(n)], axis=0)
            for k in self.in_names
        ]
        concat += [
            np.zeros((n * z.shape[0], *z.shape[1:]), z.dtype)
            for z in self.zero_outs
        ]
        sh = NamedSharding(self.mesh, PartitionSpec("core"))
        self._dev_args = [jax.device_put(a, sh) for a in concat]

    def run(self):
        outs = self.fn(*self._dev_args)
        # keep device arrays for reuse; pull results to host
        res = []
        for c in range(self.n_cores):
            res.append({
                name: np.asarray(outs[i]).reshape(
                    self.n_cores, *self.out_avals[i].shape)[c]
                for i, name in enumerate(self.out_names)
            })
        return res

    def time_iters(self, iters=20):
        import time
        import jax
        outs = self.fn(*self._dev_args)
        jax.block_until_ready(outs)
        t0 = time.perf_counter()
        for _ in range(iters):
            outs = self.fn(*self._dev_args)
        jax.block_until_ready(outs)
        t1 = time.perf_counter()
        return (t1 - t0) / iters


_RUNNER_CACHE = {}


def get_runner(S):
    if S not in _RUNNER_CACHE:
        _RUNNER_CACHE[S] = PjrtRunner(_get_program(S))
    return _RUNNER_CACHE[S]


def combine_outputs(results, W_qkv, b_qkv, W_out, b_out, B, S, D):
    b_v = np.concatenate([b_qkv[192 * h + 128:192 * h + 192] for h in range(16)])
    corr = (b_v.astype(np.float64) @ W_out.astype(np.float64)).astype(np.float32)
    corr += b_out
    out = np.zeros((B, S, D), np.float32)
    for c in range(8):
        out[c // 4] += results[c]["out"].astype(np.float32)
    out += corr[None, None, :]
    return out


def kernel(x, W_qkv, b_qkv, W_out, b_out):
    x = np.asarray(x)
    W_qkv = np.asarray(W_qkv)
    b_qkv = np.asarray(b_qkv)
    W_out = np.asarray(W_out)
    b_out = np.asarray(b_out)
    B, S, D = x.shape

    runner = get_runner(S)
    runner.stage(make_in_maps(x, W_qkv, b_qkv, W_out))
    results = runner.run()
    return combine_outputs(results, W_qkv, b_qkv, W_out, b_out, B, S, D)
